# revision 19
# baseline (speedup 1.0000x reference)
"""CoWindowAttention Trainium2 kernel — 8-core data-parallel Bass/Tile.

Feature-major layout ([feature, token] in SBUF) so PE contractions never
need on-chip transposes.  Host pre-transposes big I/O, folds W1@Wq (only q
of big is used), folds the softmax scale into the weights, moves the
epilogue bias (bv@W2+b2) to the host-side gather, and ships exp(rel-pos
bias) so the kernel multiplies instead of add-then-exp.

Scores are computed transposed (k-token on partitions), window-parity
packed: score tile t holds heads 2t,2t+1 at rows 64g+k, cols (h%2)*256+q.
All matmuls run in plain 128x128 array mode at tile_position (0,0) (this
runtime crashes on mixed PE tile positions/modes); small contractions are
zero-padded to K=128, which is free since PE time is N-bound.  All matmul
operands are bf16 (FWL weight loads, half DMA); PSUM accumulation is f32.

The group loop is software-pipelined 3 deep (A: load+project, B: scores+
exp, C: softmax-normalize+output) with stage emissions interleaved so each
engine always has ready work and the PE stays HAM-warm.
"""

import sys
import numpy as np

if "/opt/trn_rl_repo" not in sys.path:
    sys.path.insert(0, "/opt/trn_rl_repo")

from contextlib import ExitStack

from concourse import bacc, bass, tile, mybir
from concourse.bass_utils import run_bass_kernel_spmd

W_, WU, H, SF, BF, HD = 8, 16, 4, 128, 256, 32
NB, NS = WU * WU, W_ * W_          # 256, 64
B, NCORES = 1024, 8
BLOC = B // NCORES
G = 2
NGRP = BLOC // G
import os as _os
NGRP_RUN = int(_os.environ.get("KNGRP", NGRP))
SCALE = HD ** -0.5

F32 = mybir.dt.float32
BF16 = mybir.dt.bfloat16
AF = mybir.ActivationFunctionType
ALU = mybir.AluOpType


def _rel_pos_index():
    ch, cw = np.meshgrid(np.arange(WU), np.arange(WU), indexing="ij")
    big = np.stack([ch.reshape(-1), cw.reshape(-1)])
    sh, sw = np.meshgrid(np.arange(W_), np.arange(W_), indexing="ij")
    small = np.stack([sh.reshape(-1), sw.reshape(-1)])
    rel = big[:, :, None] - small[:, None, :]
    return (rel[0] + W_ - 1) * (2 * W_ - 1) + (rel[1] + W_ - 1)   # (NB, NS)


def build_nc():
    nc = bacc.Bacc("TRN2", target_bir_lowering=False, debug=False,
                   enable_asserts=False)

    bigxT = nc.dram_tensor("bigxT", (BLOC, BF, NB), BF16, kind="ExternalInput").ap()
    smallxT = nc.dram_tensor("smallxT", (BLOC, SF, NS), BF16, kind="ExternalInput").ap()
    wbq_d = nc.dram_tensor("wbq", (BF, SF), BF16, kind="ExternalInput").ap()
    wk_d = nc.dram_tensor("wk", (SF, SF), BF16, kind="ExternalInput").ap()
    wv_d = nc.dram_tensor("wv", (SF, SF), BF16, kind="ExternalInput").ap()
    w2_d = nc.dram_tensor("w2", (SF, BF), BF16, kind="ExternalInput").ap()
    expb_d = nc.dram_tensor("expb", (128, 1024), BF16, kind="ExternalInput").ap()
    onesz_d = nc.dram_tensor("onesz", (128, 512), BF16, kind="ExternalInput").ap()
    ind8_d = nc.dram_tensor("ind8", (128, 256), BF16, kind="ExternalInput").ap()
    bvec_d = nc.dram_tensor("bvec", (128, 4), F32, kind="ExternalInput").ap()
    zeros_d = nc.dram_tensor("zeros", (128, 512), BF16, kind="ExternalInput").ap()
    eye_d = nc.dram_tensor("eye", (128, 128), BF16, kind="ExternalInput").ap()
    outT = nc.dram_tensor("outT", (BLOC, BF, NB), F32, kind="ExternalOutput").ap()

    with ExitStack() as ctx:
        ctx.enter_context(nc.allow_low_precision(reason="bf16 matmul inputs"))
        tc = ctx.enter_context(tile.TileContext(nc))
        wp = ctx.enter_context(tc.tile_pool(name="w", bufs=1))
        sb = ctx.enter_context(tc.tile_pool(name="sb", bufs=3))
        ps = ctx.enter_context(tc.tile_pool(name="ps", bufs=1, space="PSUM"))

        wbq = wp.tile([128, 256], BF16)
        nc.sync.dma_start(wbq[:].rearrange("p (c m) -> p c m", c=2),
                          wbq_d.rearrange("(c p) m -> p c m", p=128))
        wk = wp.tile([128, 128], BF16)
        nc.sync.dma_start(wk[:], wk_d)
        wv = wp.tile([128, 128], BF16)
        nc.sync.dma_start(wv[:], wv_d)
        w2 = wp.tile([128, 256], BF16)
        nc.sync.dma_start(w2[:], w2_d)
        expb = wp.tile([128, 1024], BF16)
        nc.sync.dma_start(expb[:], expb_d)
        onesz = wp.tile([128, 512], BF16)
        nc.sync.dma_start(onesz[:], onesz_d)
        ind8 = wp.tile([128, 256], BF16)
        nc.sync.dma_start(ind8[:], ind8_d)
        bvec = wp.tile([128, 4], F32)
        nc.sync.dma_start(bvec[:], bvec_d)
        eye = wp.tile([128, 128], BF16)
        nc.sync.dma_start(eye[:], eye_d)
        # persistent zero-padded stationaries, even/odd pairs
        k_pads, v_pads, rzs = [], [], []
        for i in range(3):
            vp = wp.tile([128, 1024], BF16, name=f"v_pad{i}")
            nc.sync.dma_start(vp[:, 0:512], zeros_d)
            nc.sync.dma_start(vp[:, 512:1024], zeros_d)
            v_pads.append(vp)
        for i in range(2):
            kp = wp.tile([128, 1024], BF16, name=f"k_pad{i}")
            nc.sync.dma_start(kp[:, 0:512], zeros_d)
            nc.sync.dma_start(kp[:, 512:1024], zeros_d)
            k_pads.append(kp)

            rzt = wp.tile([128, 256], BF16, name=f"rz{i}")
            nc.sync.dma_start(rzt[:], zeros_d[:, 0:256])
            rzs.append(rzt)

        st = {}   # per-group in-flight state

        def stage_a(i):
            """load + project q/k/v for group i"""
            b0 = i * G
            k_pad, v_pad = k_pads[i % 2], v_pads[i % 3]
            big = sb.tile([128, G * 512], BF16, tag="big", name=f"big{i}")
            for c in range(2):
                nc.sync.dma_start(
                    big[:, c * 512:(c + 1) * 512].rearrange("p (g n) -> p g n", g=G),
                    bigxT[b0:b0 + G, c * 128:(c + 1) * 128, :].rearrange("g p n -> p g n"))
            small = sb.tile([128, G * 64], BF16, tag="small", name=f"small{i}")
            nc.sync.dma_start(
                small[:].rearrange("p (g n) -> p g n", g=G),
                smallxT[b0:b0 + G].rearrange("g p n -> p g n"))

            qb_ps = ps.tile([128, 512], F32, tag="qb_ps", bufs=1, name=f"qbp{i}")
            for c in range(2):
                nc.tensor.matmul(qb_ps[:], wbq[:, c * 128:(c + 1) * 128],
                                 big[:, c * 512:(c + 1) * 512],
                                 start=(c == 0), stop=(c == 1))
            qb = sb.tile([128, 512], BF16, tag="qb", name=f"qb{i}")
            nc.scalar.activation(qb[:], qb_ps[:], AF.Identity, bias=bvec[:, 0:1])

            kv_ps = ps.tile([128, 256], F32, tag="kv_ps", bufs=1, name=f"kvp{i}")
            nc.tensor.matmul(kv_ps[:, 0:128], wk[:], small[:], start=True, stop=True)
            nc.tensor.matmul(kv_ps[:, 128:256], small[:], wv[:], start=True, stop=True)
            # scatter k into k_pad block 2h+g (rows 32h, col 256h+192g+n)
            for h in range(4):
                nc.scalar.activation(
                    bass.AP(k_pad.tensor, k_pad.offset + 32 * h * 1024 + 256 * h,
                            [[1024, 32], [192, 2], [1, 64]]),
                    kv_ps[32 * h:32 * h + 32, 0:128].rearrange("p (g n) -> p g n", g=2),
                    AF.Identity, bias=bvec[32 * h:32 * h + 32, 1:2])
            # scatter v into v_pad block 4g+h (rows 64g, col 512g+160h+d), DVE
            for g in range(G):
                nc.vector.tensor_copy(
                    bass.AP(v_pad.tensor, v_pad.offset + 64 * g * 1024 + 512 * g,
                            [[1024, 64], [160, 4], [1, 32]]),
                    kv_ps[64 * g:64 * g + 64, 128:256].rearrange("p (h d) -> p h d", h=4))
            st[i] = dict(qb=qb, k_pad=k_pad, v_pad=v_pad)

        def stage_b(i):
            """scores + exp for group i"""
            g_ = st[i]
            qb, k_pad = g_["qb"], g_["k_pad"]
            es_t = []
            for t in range(2):
                sp = ps.tile([128, 512], F32, tag="sp", bufs=3, name=f"s{t}_{i}")
                nc.tensor.matmul(sp[:], eye[:], expb[:, t * 512:(t + 1) * 512],
                                 start=True, stop=False, skip_group_check=True)
                for h2 in range(2):
                    h = 2 * t + h2
                    for g in range(G):
                        blk = 2 * h + g
                        nc.tensor.matmul(
                            sp[:, h2 * 256:h2 * 256 + 256],
                            k_pad[:, 128 * blk:128 * (blk + 1)],
                            qb[:, g * 256:(g + 1) * 256],
                            start=False, stop=(g == 1), skip_group_check=True)
                es = sb.tile([128, 512], BF16, tag=f"es{t}", name=f"es{t}_{i}")
                nc.scalar.activation(es[:], sp[:], AF.Exp)
                es_t.append(es)
            g_["es_t"] = es_t

        def stage_c_z(i):
            """softmax denominators for group i (PE, needs es from B(i))"""
            g_ = st[i]
            es_t = g_["es_t"]
            z_ps = ps.tile([128, 256], F32, tag="zru", bufs=3, name=f"z{i}")
            for t in range(2):
                for half in range(2):
                    j = 2 * t + half
                    nc.tensor.matmul(z_ps[:], onesz[:, 128 * j:128 * (j + 1)],
                                     es_t[t][:, half * 256:half * 256 + 256],
                                     start=(j == 0), stop=(j == 3))
            g_["z_ps"] = z_ps

        def stage_c(i):
            """normalize + final projection + output for group i"""
            g_ = st.pop(i)
            es_t, v_pad, z_ps, rz = g_["es_t"], g_["v_pad"], g_["z_ps"], rzs[i % 2]
            b0 = i * G
            nc.vector.reciprocal(rz[0:8, :], z_ps[0:8, :])
            rzb_ps = ps.tile([128, 512], F32, tag="zru", bufs=3, name=f"rzb{i}")
            for g in range(G):
                nc.tensor.matmul(rzb_ps[:, g * 256:(g + 1) * 256],
                                 ind8[:, g * 128:(g + 1) * 128], rz[:],
                                 start=True, stop=True)
            u_ps = ps.tile([128, 512], F32, tag="zru", bufs=3, name=f"u{i}")
            for g in range(G):
                for h in range(4):
                    blk = 4 * g + h
                    nc.tensor.matmul(
                        u_ps[:, g * 256:(g + 1) * 256],
                        v_pad[:, 128 * blk:128 * (blk + 1)],
                        es_t[h // 2][:, (h % 2) * 256:(h % 2) * 256 + 256],
                        start=(h == 0), stop=(h == 3))
            rzb_sb = sb.tile([128, 512], F32, tag="rzb", name=f"rzbs{i}")
            nc.vector.tensor_copy(rzb_sb[:], rzb_ps[:])
            un = sb.tile([128, 512], BF16, tag="un", name=f"un{i}")
            nc.vector.tensor_tensor(un[:], u_ps[:], rzb_sb[:], ALU.mult)
            out_sb = sb.tile([128, 1024], F32, tag="out", name=f"out{i}")
            for c in range(2):
                p_ps = ps.tile([128, 512], F32, tag="sp", bufs=3, name=f"p{c}_{i}")
                nc.tensor.matmul(p_ps[:], w2[:, c * 128:(c + 1) * 128], un[:],
                                 start=True, stop=True)
                if c == 0:
                    nc.scalar.activation(out_sb[:, 0:512], p_ps[:], AF.Identity)
                else:
                    nc.vector.tensor_copy(out_sb[:, 512:1024], p_ps[:])
                nc.sync.dma_start(
                    outT[b0:b0 + G, c * 128:(c + 1) * 128, :].rearrange("g p n -> p g n"),
                    out_sb[:, c * 512:(c + 1) * 512].rearrange("p (g n) -> p g n", g=G))

        # software pipeline: A(i+2) | Z(i) | B(i+1) | C(i)
        stage_a(0)
        if NGRP_RUN > 1:
            stage_a(1)
        stage_b(0)
        for i in range(NGRP_RUN):
            if i + 2 < NGRP_RUN:
                stage_a(i + 2)
            stage_c_z(i)
            if i + 1 < NGRP_RUN:
                stage_b(i + 1)
            stage_c(i)

    nc.compile()
    return nc


_NC = None


def _get_nc():
    global _NC
    if _NC is None:
        _NC = build_nc()
    return _NC


def _host_consts(W1, b1, Wqkv, bqkv, W2, b2, bias_table):
    import ml_dtypes
    BFnp = ml_dtypes.bfloat16
    Wq, Wk, Wv = Wqkv[:, :SF], Wqkv[:, SF:2 * SF], Wqkv[:, 2 * SF:]
    bq, bk, bv = bqkv[:SF], bqkv[SF:2 * SF], bqkv[2 * SF:]
    wbq = (W1 @ Wq) * SCALE
    bbq = (b1 @ Wq + bq) * SCALE
    c2 = (bv @ W2 + b2).astype(np.float32)
    bias = bias_table[_rel_pos_index()]            # (NB, NS, H)
    biasT = np.zeros((128, 1024), np.float32)
    for h in range(H):
        bT = bias[:, :, h].T
        for g in range(G):
            biasT[64 * g:64 * g + 64,
                  (h // 2) * 512 + (h % 2) * 256:
                  (h // 2) * 512 + (h % 2) * 256 + 256] = bT
    onesz = np.zeros((128, 512), np.float32)
    for j in range(4):
        for g in range(G):
            onesz[64 * g:64 * g + 64, 128 * j + 2 * j + g] = 1.0
    ind8 = np.zeros((128, 256), np.float32)
    for h in range(H):
        for g in range(G):
            ind8[2 * h + g, g * 128 + 32 * h:g * 128 + 32 * h + 32] = 1.0
    bvec = np.zeros((128, 4), np.float32)
    bvec[:, 0] = bbq
    bvec[:, 1] = bk
    consts = dict(wbq=np.ascontiguousarray(wbq.astype(BFnp)),
                  wk=np.ascontiguousarray(Wk.astype(BFnp)),
                  wv=np.ascontiguousarray(Wv.astype(BFnp)),
                  w2=np.ascontiguousarray(W2.astype(BFnp)),
                  expb=biasT.astype(BFnp),
                  onesz=onesz.astype(BFnp), ind8=ind8.astype(BFnp),
                  bvec=bvec, zeros=np.zeros((128, 512), BFnp),
                  eye=np.eye(128).astype(BFnp))
    return consts, c2


def make_in_maps(big_x, small_x, W1, b1, Wqkv, bqkv, W2, b2, bias_table):
    import ml_dtypes
    BFnp = ml_dtypes.bfloat16
    consts, c2 = _host_consts(
        np.asarray(W1, np.float32), np.asarray(b1, np.float32),
        np.asarray(Wqkv, np.float32), np.asarray(bqkv, np.float32),
        np.asarray(W2, np.float32), np.asarray(b2, np.float32),
        np.asarray(bias_table, np.float32))
    big_x = np.asarray(big_x, np.float32)
    small_x = np.asarray(small_x, np.float32)
    in_maps = []
    for c in range(NCORES):
        sl = slice(c * BLOC, (c + 1) * BLOC)
        m = dict(consts)
        m["bigxT"] = np.ascontiguousarray(big_x[sl].transpose(0, 2, 1).astype(BFnp))
        m["smallxT"] = np.ascontiguousarray(small_x[sl].transpose(0, 2, 1).astype(BFnp))
        in_maps.append(m)
    return in_maps, c2


def gather_out(results, c2):
    # outT is (BLOC, BF, NB); epilogue bias c2 (per BF feature) added here
    outs = [(r["outT"] + c2[None, :, None]).transpose(0, 2, 1) for r in results]
    return np.ascontiguousarray(np.concatenate(outs, axis=0), dtype=np.float32)


def run(inputs, **kw):
    nc = _get_nc()
    in_maps, c2 = make_in_maps(**inputs)
    res = run_bass_kernel_spmd(nc, in_maps, core_ids=list(range(NCORES)), **kw)
    res.c2 = c2
    return res


def kernel(**inputs):
    res = run(inputs)
    return gather_out(res.results, res.c2)
